# revision 4
# baseline (speedup 1.0000x reference)
"""Bilinear interaction kernel for Trainium2 (8 NeuronCores, SPMD) — v2.

Problem: inputs (32, 4096, 1, 64) f32 stacked field embeddings,
W (496, 64, 64) one bilinear weight per field pair (i<j).
out[b, p] = x_i[b] @ W_p @ x_j[b]   -> (4096, 496) f32.

v2 strategy (vs the f32 baseline, which was DVE-bound at ~330us):
 - all inputs bf16 (error << the 2e-2 gate): stage-1 matmuls run at
   1 cyc/row instead of f32's 4.
 - stage-1 (PE): per first-field i, T[b,(j,l)] = x_i @ W_i in psum
   units of <=1024 cols (2 banks).
 - stage-2 MUL (M = T * xn) routed per unit across engines:
     'p'  : DVE tensor_mul straight from PSUM (f32 x bf16 -> bf16)
     'ag' : ACT copies psum -> bf16 SBUF, GPSIMD multiplies
     'a'  : ACT copies, DVE multiplies (bf16 2x mode)
 - stage-3 RED (sum over l=64) routed per pair-region:
     PE   : 64 accumulating identity matmuls per region into a psum bank
     DVE  : pairwise bf16 tree adds (4x tensor_tensor mode)
     GP   : pairwise tree adds
 - walrus allows ONE sync wait per instruction: touch ops absorb
   foreign-engine ticks into each engine's observed vector clock
   (transitively), so every real instruction needs at most one wait.
"""

import os
import sys

import numpy as np

sys.path.insert(0, "/opt/trn_rl_repo")

import concourse.bass as bass
import concourse.tile as tile
from concourse import mybir
from concourse.bass_utils import run_bass_kernel_spmd
from concourse.tile import ScopedClock


def _split_drain_and_barrier(self, tick_clock, wait_clock):
    """walrus accepts only one sync wait per instruction; the kernel-tail
    drain collects one wait per active processor. Emit one drain per wait."""
    drains = [self.nc.sync.drain() for _ in range(20)]
    wait_clock.add_sem_waits(
        drains[-1].ins, ScopedClock({None: tick_clock.global_clock})
    )
    si = drains[-1].ins.sync_info
    ow = list(si.on_wait) if si is not None and si.on_wait else []
    if len(ow) > 1:
        for d, w in zip(drains[:-1], ow[:-1]):
            d.ins.sync_info = mybir.SyncInfo(on_wait=[w], on_update=[])
        drains[-1].ins.sync_info = mybir.SyncInfo(
            on_wait=[ow[-1]],
            on_update=list(si.on_update) if si.on_update else [],
        )

    self.nc.all_engine_barrier()
    assert self.sems is not None
    popped = self.nc._tile_sem_poison_stack.pop()
    assert popped is self._sem_poison
    self.nc.clear_and_free_semaphores(list(self.sems.allocated().values()))
    self.nc.all_engine_barrier()


tile.TileContext._drain_and_barrier = _split_drain_and_barrier

# the identity-reduce bursts reload the same stationary every matmul; let
# walrus elide the redundant LDWEIGHTS
import concourse.bass_utils as _bu

_orig_walrus_args = _bu.get_walrus_args


def _walrus_args_ldwopt(*a, **kw):
    args = _orig_walrus_args(*a, **kw)
    return [
        x.replace("--enable-ldw-opt=false", "--enable-ldw-opt=true")
        if isinstance(x, str) else x
        for x in args
    ]


_bu.get_walrus_args = _walrus_args_ldwopt

_ENG_ATTR = {
    mybir.EngineType.PE: "tensor",
    mybir.EngineType.DVE: "vector",
    mybir.EngineType.Activation: "scalar",
    mybir.EngineType.Pool: "gpsimd",
    mybir.EngineType.SP: "sync",
}


def _fix_dma_waits(nc: bass.Bass):
    """SP-queue DMA triggers with engine-sem waits fail at NEFF load on
    this toolchain (queue sems like DMAHW* are fine). Move engine-sem
    waits onto a preceding sync drain (the proven wait-carrier)."""
    for f in nc.m.functions:
        for bb in f.blocks:
            il = bb.instructions
            out = []
            changed = False
            for ins in il:
                si = ins.sync_info
                ow = list(si.on_wait) if si is not None and si.on_wait else []
                eng_waits = [w for w in ow
                             if not w.ant_name.startswith("DMA")]
                if (ins.opcode == "DMACopy" and eng_waits
                        and ins.engine == mybir.EngineType.SP):
                    changed = True
                    keep = [w for w in ow if w.ant_name.startswith("DMA")]
                    for w in eng_waits:
                        d = nc.sync.drain()
                        d.ins.sync_info = mybir.SyncInfo(
                            on_wait=[w], on_update=[]
                        )
                        tail_bb = nc.cur_bb.bb
                        tl = tail_bb.instructions
                        assert tl[-1].name == d.ins.name
                        tail_bb.instructions = tl[:-1]
                        out.append(d.ins)
                    ins.sync_info = mybir.SyncInfo(
                        on_wait=keep,
                        on_update=list(si.on_update) if si.on_update else [],
                    )
                out.append(ins)
            if changed:
                bb.instructions = out


def _split_multi_waits(nc: bass.Bass):
    """Safety net: walrus accepts one sync wait per instruction. For any
    instruction still carrying several, prepend same-engine nops each
    carrying one of the extra waits (in-order engines honor them before
    the real instruction issues)."""
    for f in nc.m.functions:
        for bb in f.blocks:
            il = bb.instructions
            out = []
            changed = False
            for ins in il:
                si = ins.sync_info
                ow = list(si.on_wait) if si is not None and si.on_wait else []
                if len(ow) > 1:
                    changed = True
                    eng = getattr(nc, _ENG_ATTR[ins.engine])
                    for w in ow[:-1]:
                        nop = eng.nop()
                        nop.ins.sync_info = mybir.SyncInfo(
                            on_wait=[w], on_update=[]
                        )
                        # the nop was appended to the current (tail) bb;
                        # move it here instead
                        tail_bb = nc.cur_bb.bb
                        tl = tail_bb.instructions
                        assert tl[-1].name == nop.ins.name
                        tail_bb.instructions = tl[:-1]
                        out.append(nop.ins)
                    ins.sync_info = mybir.SyncInfo(
                        on_wait=[ow[-1]],
                        on_update=list(si.on_update) if si.on_update else [],
                    )
                out.append(ins)
            if changed:
                bb.instructions = out

NF = 32          # fields
B = 4096         # total batch
K = 64           # embedding dim
P = NF * (NF - 1) // 2   # 496 pairs
NCORES = 8
BC = B // NCORES          # 512 rows per core
BT = 128                  # batch tile (partition dim)
NBT = BC // BT            # 4 batch tiles per core
F32 = mybir.dt.float32
BF16 = mybir.dt.bfloat16
UNIT = int(os.environ.get("V2_UNIT", "1024"))  # psum unit width
_PETOUCH = os.environ.get("V2_PETOUCH", "0") == "1"
_TB_BUFS = int(os.environ.get("V2_TB_BUFS", "8"))
_A_UNITS = int(os.environ.get("V2_A_UNITS", "0"))
_TREE2PE = os.environ.get("V2_TREE2PE", "0") == "1"
_SPLITRED = os.environ.get("V2_SPLITRED", "0") == "1"
_AGTOUCH = os.environ.get("V2_AGTOUCH", "1") == "1"
_ACTMT = os.environ.get("V2_ACTMT", "1") == "1"
_INLOOP_LD = os.environ.get("V2_INLOOP_LD", "1") == "1"
_NOGPMUL = os.environ.get("V2_NOGPMUL", "0") == "1"
_ONLY_T0 = os.environ.get("V2_ONLY_T0", "0") == "1"
_NORED = os.environ.get("V2_NORED", "0") == "1"
_MAXU = int(os.environ.get("V2_MAXU", "999"))
_SYNC_OUT = os.environ.get("V2_SYNC_OUT", "0") == "1"
_MAXW = int(os.environ.get("V2_MAXW", "999"))
_RED_KINDS = set(os.environ.get("V2_RED_KINDS", "pe,dv,gp").split(","))
_NODRAIN = os.environ.get("V2_NODRAIN", "0") == "1"
_NIDENT = int(os.environ.get("V2_NIDENT", "64"))
_TDEPTH = int(os.environ.get("V2_TDEPTH", "6"))
_NOGPTREE = os.environ.get("V2_NOGPTREE", "0") == "1"

# pair-group column offsets (pairs ordered like itertools.combinations)
_GRP_OFF = [0] * NF
for _i in range(1, NF):
    _GRP_OFF[_i] = _GRP_OFF[_i - 1] + (NF - _i)

# ---- field -> (xt tile, slot, half) packing --------------------------------
# top half (partitions 0:64): fields 0-7 (tile A slots 0-7) and 24-31 (tile B)
# bottom half (64:128):       fields 8-15 (tile A) and 16-23 (tile B)


def _field_loc(i: int):
    if i < 8:
        return 0, i, 0        # tile A, slot i, top
    if i < 16:
        return 0, i - 8, 1    # tile A, bottom
    if i < 24:
        return 1, i - 16, 1   # tile B, bottom
    return 1, i - 24, 0       # tile B, top


_XTP_FIELD = np.zeros((2, 16), dtype=np.int64)   # [half, tile*8+slot] -> field
for _i in range(NF):
    _tl, _sl, _hf = _field_loc(_i)
    _XTP_FIELD[_hf, _tl * 8 + _sl] = _i

# ---- stage-1 units and W block packing -------------------------------------
# unit = (i, off, w): psum tile of w<=1024 T columns for first-field i
_UNITS_TOP = []
_UNITS_BOT = []
for _i in range(NF - 1):
    _ncols = (NF - 1 - _i) * K
    _lst = _UNITS_TOP if _field_loc(_i)[2] == 0 else _UNITS_BOT
    for _off in range(0, _ncols, UNIT):
        _lst.append((_i, _off, min(UNIT, _ncols - _off)))
_UNITS_TOP.sort(key=lambda c: (-c[2], c[0], c[1]))
_UNITS_BOT.sort(key=lambda c: (-c[2], c[0], c[1]))
assert len(_UNITS_TOP) == len(_UNITS_BOT)
_NBLK = len(_UNITS_TOP)

# W blocks: block j pairs _UNITS_TOP[j] (rows 0:64) with _UNITS_BOT[j]
# (rows 64:128). Width multisets match exactly, so no padding.
_WBLK = []        # (top_unit, bot_unit, width)
_WCOL = []        # start col of block j in packed W
_c = 0
for _j in range(_NBLK):
    _w = max(_UNITS_TOP[_j][2], _UNITS_BOT[_j][2])
    _WBLK.append((_UNITS_TOP[_j], _UNITS_BOT[_j], _w))
    _WCOL.append(_c)
    _c += _w
_WCOLS = _c

# execution order: alternate top/bottom so stage-1 row-packing overlaps
_ORDER = []       # (unit, blk_idx, half)
for _j in range(_NBLK):
    _ORDER.append((_UNITS_TOP[_j], _j, 0))
    _ORDER.append((_UNITS_BOT[_j], _j, 1))

# ---- routing and RED regions (tunable) -------------------------------------
# route per unit: 'p' (DVE from psum), 'ag' (ACT copy + GP mul),
#                 'a' (ACT copy + DVE mul)
# default: top units 'p', bottom units 'ag'
_ROUTE = {}
for _u in _UNITS_TOP:
    _ROUTE[_u] = "p"
for _u in _UNITS_BOT:
    _ROUTE[_u] = "a" if _NOGPMUL else "ag"
# offload a few top units to ACT-copy + DVE-bf16-mul (DVE is the
# saturated engine; the bf16 2x mode halves its per-col cost)
for _k in range(_A_UNITS):
    _ROUTE[_UNITS_TOP[8 + _k]] = "a"

# RED regions by field boundary. PE region must cover exactly the top pairs
# (their M columns are written by DVE 'p' MULs whose psum wait subsumes the
# cross-b-tile WAR on PE).
# top pairs: fields 0-7 -> pairs 0..219, fields 24-30 -> pairs 468..495.
_DG = 123   # DVE-tree pairs total (fields 8-13)
_GG = 125   # GP-tree pairs total (fields 14-23)

# emission indexes where each RED becomes ready: last ORDER index covering
# the region's fields
_last_cover = {}
for _idx, ((_i, _off, _w), _j, _hf) in enumerate(_ORDER):
    _last_cover[_i] = _idx


def _region_ready(fields):
    return max(_last_cover[f] for f in fields if f in _last_cover)


# RED plan: (ready_idx, kind, pair0, npairs, dstcol/scr_off)
#  kind 'pe': ident tile into acc_ps at col dstcol
#  kind 'dv'/'gp': tree over pairs [pair0, pair0+npairs) using the engine's
#  scratch rows [scr_off, scr_off+npairs)
if _SPLITRED:
    _RED_PLAN = [
        (_region_ready(range(0, 5)), "pe", 0, 124, 0),
        (_region_ready(range(5, 8)), "pe", 124, 96, 124),
        (_region_ready(range(24, 31)), "pe", 468, 28, 220),
        (_region_ready(range(8, 11)), "dv", 220, 66, 0),
        (_region_ready(range(11, 14)), "pe" if _TREE2PE else "dv", 286, 57,
         248 if _TREE2PE else 66),
        (_region_ready(range(14, 19)), "gp", 343, 75, 0),
        (_region_ready(range(19, 24)), "gp", 418, 50, 75),
    ]
else:
    _RED_PLAN = [
        (_region_ready([i for i in range(NF - 1)
                        if _field_loc(i)[2] == 0]), "pe", 0, 124, 0),
        (_region_ready([i for i in range(NF - 1)
                        if _field_loc(i)[2] == 0]), "pe", 124, 96, 124),
        (_region_ready([i for i in range(NF - 1)
                        if _field_loc(i)[2] == 0]), "pe", 468, 28, 220),
        (_region_ready(range(8, 14)), "pe" if _TREE2PE else "dv", 286, 57,
         248 if _TREE2PE else 66),
        (_region_ready(range(8, 14)), "dv", 220, 66, 0),
        (_region_ready(range(14, 24)), "dv" if _NOGPTREE else "gp", 343, 75, 123),
        (_region_ready(range(14, 24)), "dv" if _NOGPTREE else "gp", 418, 50, 198),
    ]
_RED_PLAN = sorted(_RED_PLAN, key=lambda r: r[0])


def _build_module() -> bass.Bass:
    nc = bass.Bass()
    xn = nc.declare_dram_parameter("xn", [BC, NF * K], BF16, isOutput=False)
    xtp = nc.declare_dram_parameter("xtp", [BT, 16, BC], BF16, isOutput=False)
    wt = nc.declare_dram_parameter("wt", [BT, _WCOLS], BF16, isOutput=False)
    ident = nc.declare_dram_parameter("ident", [BT, BT], BF16, isOutput=False)
    outs = [
        nc.declare_dram_parameter(f"out{t}", [BT, P], F32, isOutput=True)
        for t in range(NBT)
    ]

    with tile.TileContext(nc) as tc:
        with (
            tc.tile_pool(name="wpool", bufs=1) as wpool,
            tc.tile_pool(name="xnp", bufs=1) as xnp,
            tc.tile_pool(name="xtp_pool", bufs=1) as xtpool,
            tc.tile_pool(name="mpool", bufs=1) as mpool,
            tc.tile_pool(name="tbp", bufs=_TB_BUFS) as tbp,
            tc.tile_pool(name="scrp", bufs=1) as scrp,
            tc.tile_pool(name="accp", bufs=1) as accp,
            tc.tile_pool(name="ups", bufs=(3 if UNIT == 1024 else 6), space=bass.MemorySpace.PSUM) as ups,
            tc.tile_pool(name="accps", bufs=1, space=bass.MemorySpace.PSUM) as accpsp,
            tc.tile_pool(name="sinkp", bufs=1, space=bass.MemorySpace.PSUM) as sinkp,
        ):
            sink = sinkp.tile([BT, 8], F32)
            junk = wpool.tile([BT, 256], F32, tag="junk")
            junka = wpool.tile([BT, 256], F32, tag="junka")
            junkg = wpool.tile([BT, 256], F32, tag="junkg")
            _tcnt = {"d": 0, "a": 0, "g": 0}

            _NLT = os.environ.get("V2_NO_LOAD_TOUCH", "0") == "1"

            def pe_touch(ap2d):
                if _NLT:
                    return
                # 1-col bf16 matmuls crash the NEFF on real TRN2; touch via
                # an f32 view of the same bytes instead
                a = (ap2d[:, 0:2].bitcast(F32)
                     if ap2d.dtype == BF16 else ap2d[:, 0:1])
                nc.tensor.matmul(
                    sink[0:1, 0:1], a[:, 0:1], a[:, 0:1],
                    start=True, stop=True,
                )

            def dve_touch(ap2d):
                c = _tcnt["d"] % 256
                _tcnt["d"] += 1
                nc.vector.tensor_copy(junk[0:1, c:c + 1], ap2d[0:1, 0:1])

            def act_touch(ap2d):
                c = _tcnt["a"] % 256
                _tcnt["a"] += 1
                nc.scalar.copy(junka[0:1, c:c + 1], ap2d[0:1, 0:1])

            def gp_touch(ap2d):
                c = _tcnt["g"] % 256
                _tcnt["g"] += 1
                nc.gpsimd.tensor_copy(junkg[0:1, c:c + 1], ap2d[0:1, 0:1])

            # persistent M buffer [BT, P*K] bf16, reused across b-tiles
            M = mpool.tile([BT, P * K], BF16, tag="M")
            M3 = M[:].rearrange("p (a b) -> p a b", b=K)
            if _MAXU < 999 and os.environ.get("V2_MEMSET", "1") == "1":  # M bf16 memset
                nc.vector.memset(M[:], 0.0)

            # tree scratch, one set per engine, reused across b-tiles
            _dgr = 248 if _NOGPTREE else _DG
            scrD = [
                scrp.tile([BT, _dgr, w], BF16, tag=f"scrD{w}", name=f"scrD{w}")
                for w in (32, 16, 8, 4, 2)
            ]
            scrG = [
                scrp.tile([BT, _GG, w], BF16, tag=f"scrG{w}", name=f"scrG{w}")
                for w in (32, 16, 8, 4, 2)
            ]
            treedump = [scrp.tile([BT, 256], F32, tag="treedump",
                                  name="treedump")]

            # ---- loads -----------------------------------------------------
            xn_tiles = [None] * NBT
            xt_tiles = [None] * NBT

            def load_xn(t, q=nc.sync):
                xn_sb = xnp.tile([BT, NF * K], BF16, tag=f"xn{t}", name="xn_sb")
                q.dma_start(xn_sb[:], xn[t * BT:(t + 1) * BT, :])
                dve_touch(xn_sb)
                gp_touch(xn_sb)
                xn_tiles[t] = xn_sb

            def load_xt(t, tl, q=nc.sync):
                xg = xtpool.tile(
                    [BT, 8, BT], BF16, tag=f"xt{t}_{tl}", name="xg"
                )
                q.dma_start(
                    xg[:], xtp[:, 8 * tl: 8 * (tl + 1), t * BT:(t + 1) * BT]
                )
                pe_touch(xg[0:64, 0, :])
                pe_touch(xg[64:128, 0, :])
                if xt_tiles[t] is None:
                    xt_tiles[t] = [None, None]
                xt_tiles[t][tl] = xg

            w_tiles = [None] * len(_WBLK)

            def load_w(j):
                ct, cb, w = _WBLK[j]
                wtile = wpool.tile([BT, w], BF16, tag=f"w{j}", name="wtile")
                nc.sync.dma_start(wtile[:], wt[:, _WCOL[j]: _WCOL[j] + w])
                pe_touch(wtile[0:64, :])
                pe_touch(wtile[64:128, :])
                w_tiles[j] = wtile

            ident_sb = wpool.tile([BT, BT], BF16, tag="ident")

            # b-tile 0's working set races the first compute on the sync
            # queue; everything else streams on the scalar-engine queue.
            load_xt(0, 0)
            load_w(0)
            load_xn(0)
            load_xt(0, 1)
            nc.sync.dma_start(ident_sb[:], ident[:])
            pe_touch(ident_sb)
            if not _INLOOP_LD:
                for j in range(1, 6):
                    load_w(j)
                load_xn(1)
                load_xt(1, 0)
                load_xt(1, 1)
                for j in range(6, 13):
                    load_w(j)
                load_xn(2)
                load_xt(2, 0)
                load_xt(2, 1)
                for j in range(13, 19):
                    load_w(j)
                load_xn(3)
                load_xt(3, 0)
                load_xt(3, 1)
                for j in range(19, len(_WBLK)):
                    load_w(j)
            else:
                for j in range(1, min(_MAXW, len(_WBLK))):
                    load_w(j)

            # ---- compute ---------------------------------------------------
            def emit_tree(add_fn, scr, so, p0, G, dst):
                src = M3[:, p0:p0 + G, :]
                s = [sc[:, so:so + G, :] for sc in scr]
                steps = [
                    lambda: add_fn(s[0], src[:, :, 0:32], src[:, :, 32:64]),
                    lambda: add_fn(s[1], s[0][:, :, 0:16], s[0][:, :, 16:32]),
                    lambda: add_fn(s[2], s[1][:, :, 0:8], s[1][:, :, 8:16]),
                    lambda: add_fn(s[3], s[2][:, :, 0:4], s[2][:, :, 4:8]),
                    lambda: add_fn(s[4], s[3][:, :, 0:2], s[3][:, :, 2:4]),
                ]
                with nc.allow_low_precision(reason="bf16 tree; 2e-2 gate"):
                    for _k in range(min(_TDEPTH, 5)):
                        steps[_k]()
                if _TDEPTH >= 6:
                    if os.environ.get("V2_TREE_DUMMY", "0") == "1":
                        dst = treedump[0][:, :dst.shape[1]]
                    out3 = dst.rearrange("p (a o) -> p a o", o=1)
                    add_fn(out3, s[4][:, :, 0:1], s[4][:, :, 1:2])

            acc_sb0 = [None]
            for t in range(NBT):
                if _ONLY_T0 and t > 0:
                    if os.environ.get("V2_SKIP_OUT123", "0") != "1":
                        if _SYNC_OUT:
                            nc.sync.dma_start(outs[t][:], acc_sb0[0][:])
                        else:
                            nc.gpsimd.dma_start(outs[t][:], acc_sb0[0][:])
                    continue
                xn_sb = xn_tiles[t]
                xt_sb = xt_tiles[t]

                acc_ps = accpsp.tile([BT, 512], F32, tag="acc", name="acc_ps")
                acc_sb = scrp.tile([BT, P], F32, tag=f"accsb{t}", name="acc_sb")
                acc_sb0[0] = acc_sb

                copies = 0
                last_gp_mcol = [None]      # recent GP-written M elem
                cons_hist = []             # per-unit consumer loc (psum WAR)

                for idx, (u, j, half) in enumerate(_ORDER):
                    # stream the next b-tile's inputs on the sync queue
                    # (empty once the W blocks are in)
                    if _INLOOP_LD and t + 1 < NBT and idx in (12, 22, 32):
                        if idx == 12:
                            load_xn(t + 1)
                        elif idx == 22:
                            load_xt(t + 1, 0)
                        else:
                            load_xt(t + 1, 1)
                    if idx >= _MAXU:
                        continue
                    i, off, w = u
                    tl, slot, hf = _field_loc(i)
                    assert hf == half
                    pb = 64 * half
                    # absorb the psum WAR tick (consumer of unit idx-3) into
                    # PE's clock so the S1 matmuls keep only their self-wait
                    if _PETOUCH and len(cons_hist) >= 3:
                        pe_touch(cons_hist[-3])
                    ps = ups.tile([BT, UNIT], F32, name="ps")
                    for s in range(0, w, 512):
                        sw = min(512, w - s)
                        nc.tensor.matmul(
                            ps[:, s:s + sw],
                            xt_sb[tl][pb:pb + 64, slot, :],
                            w_tiles[j][pb:pb + 64, s:s + sw],
                            start=True,
                            stop=True,
                        )
                    mcol = _GRP_OFF[i] * K + off
                    c1 = (i + 1) * K + off
                    route = _ROUTE[u]
                    if route == "p":
                        nc.vector.tensor_mul(
                            M[:, mcol:mcol + w], ps[:, :w], xn_sb[:, c1:c1 + w]
                        )
                        cons_hist.append(M[:, mcol:mcol + 1])
                    else:
                        tb = tbp.tile([BT, UNIT], BF16, name="tb")
                        if (_ACTMT and copies % 3 == 2
                                and last_gp_mcol[0] is not None):
                            act_touch(M[:, last_gp_mcol[0]:last_gp_mcol[0] + 1])
                        copies += 1
                        nc.scalar.copy(tb[:, :w], ps[:, :w])
                        cons_hist.append(tb[:, 0:1])
                        if route == "ag":
                            if _AGTOUCH:
                                gp_touch(tb)
                            nc.gpsimd.tensor_mul(
                                M[:, mcol:mcol + w], tb[:, :w],
                                xn_sb[:, c1:c1 + w],
                            )
                            last_gp_mcol[0] = mcol
                        else:
                            if _AGTOUCH:
                                dve_touch(tb)
                            nc.vector.tensor_mul(
                                M[:, mcol:mcol + w], tb[:, :w],
                                xn_sb[:, c1:c1 + w],
                            )

                    for (ridx, kind, p0, npair, aux) in _RED_PLAN:
                        if ridx != idx or _NORED or kind not in _RED_KINDS:
                            continue
                        if kind == "pe":
                            # absorb the M RAW tick (MUL engine) so the
                            # ident matmuls keep only their psum self-wait
                            pe_touch(M[:, p0 * K:p0 * K + 2])
                            for l in range(_NIDENT):
                                nc.tensor.matmul(
                                    acc_ps[:, aux:aux + npair],
                                    ident_sb[:],
                                    M3[:, p0:p0 + npair, l],
                                    start=(l == 0),
                                    stop=(l == _NIDENT - 1),
                                )
                        elif kind == "dv":
                            emit_tree(
                                nc.vector.tensor_add, scrD, aux,
                                p0, npair, acc_sb[:, p0:p0 + npair],
                            )
                        else:
                            emit_tree(
                                nc.gpsimd.tensor_add, scrG,
                                0 if p0 == 343 else 75,
                                p0, npair, acc_sb[:, p0:p0 + npair],
                            )

                # drain PE-region psum accumulators -> acc_sb
                if _NORED or _NODRAIN:
                    if os.environ.get("V2_MEMSET", "1") == "1" and _NORED:
                        nc.vector.memset(acc_sb[:], 0.0)
                else:
                    for (_r, _kind, _p0, _np, _aux) in _RED_PLAN:
                        if _kind == "pe" and "pe" in _RED_KINDS:
                            nc.vector.tensor_copy(
                                acc_sb[:, _p0:_p0 + _np],
                                acc_ps[:, _aux:_aux + _np],
                            )

                pk = os.environ.get("V2_POKE_ACCSB", "0")
                if pk == "1":
                    nc.vector.tensor_copy(acc_sb[0:1, 0:1], junk[0:1, 0:1])
                elif pk == "act":
                    nc.scalar.copy(acc_sb[0:1, 0:1], junk[0:1, 0:1])
                elif pk == "gp":
                    nc.gpsimd.tensor_copy(acc_sb[0:1, 0:1], junk[0:1, 0:1])
                elif pk == "full":
                    nc.vector.tensor_copy(acc_sb[:], junk[:, 0:1].broadcast(1, P))
                # absorb DVE's acc_sb writes so the DMA keeps <=1 wait
                if _SYNC_OUT:
                    nc.sync.dma_start(outs[t][:], acc_sb[:])
                else:
                    if os.environ.get("V2_NO_ACCSB_TOUCH", "0") != "1":
                        gp_touch(acc_sb[:, 0:1])
                    nc.gpsimd.dma_start(outs[t][:], acc_sb[:])

    _split_multi_waits(nc)
    _fix_dma_waits(nc)
    return nc


_NC_CACHE: dict[str, bass.Bass] = {}


def _get_module() -> bass.Bass:
    if "nc" not in _NC_CACHE:
        _NC_CACHE["nc"] = _build_module()
    return _NC_CACHE["nc"]


def _make_in_maps(inputs: np.ndarray, W: np.ndarray):
    import ml_dtypes

    x = np.ascontiguousarray(np.asarray(inputs, dtype=np.float32)[:, :, 0, :])
    W = np.asarray(W, dtype=np.float32)

    # packed W: block j = [top unit | bottom unit] on partition halves
    wt_host = np.zeros((BT, _WCOLS), dtype=ml_dtypes.bfloat16)
    wt_flat = np.ascontiguousarray(W.transpose(1, 0, 2)).reshape(K, P * K)
    wt_flat = wt_flat.astype(ml_dtypes.bfloat16)
    for j, (ct, cb, w) in enumerate(_WBLK):
        for half, (i, off, cw) in ((0, ct), (1, cb)):
            base = _GRP_OFF[i] * K + off
            wt_host[64 * half: 64 * half + 64, _WCOL[j]: _WCOL[j] + cw] = \
                wt_flat[:, base: base + cw]

    ident = np.eye(BT, dtype=ml_dtypes.bfloat16)

    in_maps = []
    for c in range(NCORES):
        xs = x[:, c * BC:(c + 1) * BC, :]                      # (32, 512, 64)
        xn_host = np.ascontiguousarray(
            xs.transpose(1, 0, 2)
        ).reshape(BC, NF * K).astype(ml_dtypes.bfloat16)
        # xtp[p, slot16, b]: p<64 top fields, p>=64 bottom fields, k = p % 64
        xtp_host = np.empty((BT, 16, BC), dtype=ml_dtypes.bfloat16)
        xt_all = xs.transpose(2, 0, 1).astype(ml_dtypes.bfloat16)  # (64,32,512)
        xtp_host[0:64] = xt_all[:, _XTP_FIELD[0], :]
        xtp_host[64:128] = xt_all[:, _XTP_FIELD[1], :]
        in_maps.append(
            {"xn": xn_host, "xtp": xtp_host, "wt": wt_host, "ident": ident}
        )
    return in_maps


def kernel(inputs: np.ndarray, W: np.ndarray) -> np.ndarray:
    in_maps = _make_in_maps(inputs, W)
    nc = _get_module()
    res = run_bass_kernel_spmd(nc, in_maps, list(range(NCORES))).results
    return np.concatenate(
        [r[f"out{t}"] for r in res for t in range(NBT)], axis=0
    )


def kernel_profiled(inputs: np.ndarray, W: np.ndarray, tmpdir: str | None = None):
    """Run with NTFF tracing; returns (output, BassKernelResults)."""
    in_maps = _make_in_maps(inputs, W)
    nc = _get_module()
    br = run_bass_kernel_spmd(
        nc, in_maps, list(range(NCORES)), trace=True, tmpdir=tmpdir
    )
    out = np.concatenate(
        [r[f"out{t}"] for r in br.results for t in range(NBT)], axis=0
    )
    return out, br


# revision 5
# speedup vs baseline: 1.0294x; 1.0294x over previous
"""Bilinear interaction kernel for Trainium2 (8 NeuronCores, SPMD) — v2.

Problem: inputs (32, 4096, 1, 64) f32 stacked field embeddings,
W (496, 64, 64) one bilinear weight per field pair (i<j).
out[b, p] = x_i[b] @ W_p @ x_j[b]   -> (4096, 496) f32.

v2 strategy (vs the f32 baseline, which was DVE-bound at ~330us):
 - all inputs bf16 (error << the 2e-2 gate): stage-1 matmuls run at
   1 cyc/row instead of f32's 4.
 - stage-1 (PE): per first-field i, T[b,(j,l)] = x_i @ W_i in psum
   units of <=1024 cols (2 banks).
 - stage-2 MUL (M = T * xn) routed per unit across engines:
     'p'  : DVE tensor_mul straight from PSUM (f32 x bf16 -> bf16)
     'ag' : ACT copies psum -> bf16 SBUF, GPSIMD multiplies
     'a'  : ACT copies, DVE multiplies (bf16 2x mode)
 - stage-3 RED (sum over l=64) routed per pair-region:
     PE   : 64 accumulating identity matmuls per region into a psum bank
     DVE  : pairwise bf16 tree adds (4x tensor_tensor mode)
     GP   : pairwise tree adds
 - walrus allows ONE sync wait per instruction: touch ops absorb
   foreign-engine ticks into each engine's observed vector clock
   (transitively), so every real instruction needs at most one wait.
"""

import os
import sys

import numpy as np

sys.path.insert(0, "/opt/trn_rl_repo")

import concourse.bass as bass
import concourse.tile as tile
from concourse import mybir
from concourse.bass_utils import run_bass_kernel_spmd
from concourse.tile import ScopedClock


def _split_drain_and_barrier(self, tick_clock, wait_clock):
    """walrus accepts only one sync wait per instruction; the kernel-tail
    drain collects one wait per active processor. Emit one drain per wait."""
    drains = [self.nc.sync.drain() for _ in range(20)]
    wait_clock.add_sem_waits(
        drains[-1].ins, ScopedClock({None: tick_clock.global_clock})
    )
    si = drains[-1].ins.sync_info
    ow = list(si.on_wait) if si is not None and si.on_wait else []
    if len(ow) > 1:
        for d, w in zip(drains[:-1], ow[:-1]):
            d.ins.sync_info = mybir.SyncInfo(on_wait=[w], on_update=[])
        drains[-1].ins.sync_info = mybir.SyncInfo(
            on_wait=[ow[-1]],
            on_update=list(si.on_update) if si.on_update else [],
        )

    self.nc.all_engine_barrier()
    assert self.sems is not None
    popped = self.nc._tile_sem_poison_stack.pop()
    assert popped is self._sem_poison
    self.nc.clear_and_free_semaphores(list(self.sems.allocated().values()))
    self.nc.all_engine_barrier()


tile.TileContext._drain_and_barrier = _split_drain_and_barrier

# the identity-reduce bursts reload the same stationary every matmul; let
# walrus elide the redundant LDWEIGHTS
import concourse.bass_utils as _bu

_orig_walrus_args = _bu.get_walrus_args


def _walrus_args_ldwopt(*a, **kw):
    args = _orig_walrus_args(*a, **kw)
    return [
        x.replace("--enable-ldw-opt=false", "--enable-ldw-opt=true")
        if isinstance(x, str) else x
        for x in args
    ]


_bu.get_walrus_args = _walrus_args_ldwopt

_ENG_ATTR = {
    mybir.EngineType.PE: "tensor",
    mybir.EngineType.DVE: "vector",
    mybir.EngineType.Activation: "scalar",
    mybir.EngineType.Pool: "gpsimd",
    mybir.EngineType.SP: "sync",
}


def _fix_dma_waits(nc: bass.Bass):
    """SP-queue DMA triggers with engine-sem waits fail at NEFF load on
    this toolchain (queue sems like DMAHW* are fine). Move engine-sem
    waits onto a preceding sync drain (the proven wait-carrier)."""
    for f in nc.m.functions:
        for bb in f.blocks:
            il = bb.instructions
            out = []
            changed = False
            for ins in il:
                si = ins.sync_info
                ow = list(si.on_wait) if si is not None and si.on_wait else []
                eng_waits = [w for w in ow
                             if not w.ant_name.startswith("DMA")]
                if (ins.opcode == "DMACopy" and eng_waits
                        and ins.engine == mybir.EngineType.SP):
                    changed = True
                    keep = [w for w in ow if w.ant_name.startswith("DMA")]
                    for w in eng_waits:
                        d = nc.sync.drain()
                        d.ins.sync_info = mybir.SyncInfo(
                            on_wait=[w], on_update=[]
                        )
                        tail_bb = nc.cur_bb.bb
                        tl = tail_bb.instructions
                        assert tl[-1].name == d.ins.name
                        tail_bb.instructions = tl[:-1]
                        out.append(d.ins)
                    ins.sync_info = mybir.SyncInfo(
                        on_wait=keep,
                        on_update=list(si.on_update) if si.on_update else [],
                    )
                out.append(ins)
            if changed:
                bb.instructions = out


def _split_multi_waits(nc: bass.Bass):
    """Safety net: walrus accepts one sync wait per instruction. For any
    instruction still carrying several, prepend same-engine nops each
    carrying one of the extra waits (in-order engines honor them before
    the real instruction issues)."""
    for f in nc.m.functions:
        for bb in f.blocks:
            il = bb.instructions
            out = []
            changed = False
            for ins in il:
                si = ins.sync_info
                ow = list(si.on_wait) if si is not None and si.on_wait else []
                if len(ow) > 1:
                    changed = True
                    eng = getattr(nc, _ENG_ATTR[ins.engine])
                    for w in ow[:-1]:
                        nop = eng.nop()
                        nop.ins.sync_info = mybir.SyncInfo(
                            on_wait=[w], on_update=[]
                        )
                        # the nop was appended to the current (tail) bb;
                        # move it here instead
                        tail_bb = nc.cur_bb.bb
                        tl = tail_bb.instructions
                        assert tl[-1].name == nop.ins.name
                        tail_bb.instructions = tl[:-1]
                        out.append(nop.ins)
                    ins.sync_info = mybir.SyncInfo(
                        on_wait=[ow[-1]],
                        on_update=list(si.on_update) if si.on_update else [],
                    )
                out.append(ins)
            if changed:
                bb.instructions = out

NF = 32          # fields
B = 4096         # total batch
K = 64           # embedding dim
P = NF * (NF - 1) // 2   # 496 pairs
NCORES = 8
BC = B // NCORES          # 512 rows per core
BT = 128                  # batch tile (partition dim)
NBT = BC // BT            # 4 batch tiles per core
F32 = mybir.dt.float32
BF16 = mybir.dt.bfloat16
UNIT = int(os.environ.get("V2_UNIT", "1024"))  # psum unit width
_PETOUCH = os.environ.get("V2_PETOUCH", "0") == "1"
_TB_BUFS = int(os.environ.get("V2_TB_BUFS", "8"))
_A_UNITS = int(os.environ.get("V2_A_UNITS", "0"))
_TREE2PE = os.environ.get("V2_TREE2PE", "0") == "1"
_SPLITRED = os.environ.get("V2_SPLITRED", "0") == "1"
_AGTOUCH = os.environ.get("V2_AGTOUCH", "1") == "1"
_ACTMT = os.environ.get("V2_ACTMT", "1") == "1"
_INLOOP_LD = os.environ.get("V2_INLOOP_LD", "1") == "1"
_NOGPMUL = os.environ.get("V2_NOGPMUL", "0") == "1"
_ONLY_T0 = os.environ.get("V2_ONLY_T0", "0") == "1"
_NORED = os.environ.get("V2_NORED", "0") == "1"
_MAXU = int(os.environ.get("V2_MAXU", "999"))
_SYNC_OUT = os.environ.get("V2_SYNC_OUT", "0") == "1"
_MAXW = int(os.environ.get("V2_MAXW", "999"))
_RED_KINDS = set(os.environ.get("V2_RED_KINDS", "pe,dv,gp").split(","))
_NODRAIN = os.environ.get("V2_NODRAIN", "0") == "1"
_NIDENT = int(os.environ.get("V2_NIDENT", "64"))
_TDEPTH = int(os.environ.get("V2_TDEPTH", "6"))
_NOGPTREE = os.environ.get("V2_NOGPTREE", "0") == "1"

# pair-group column offsets (pairs ordered like itertools.combinations)
_GRP_OFF = [0] * NF
for _i in range(1, NF):
    _GRP_OFF[_i] = _GRP_OFF[_i - 1] + (NF - _i)

# ---- field -> (xt tile, slot, half) packing --------------------------------
# top half (partitions 0:64): fields 0-7 (tile A slots 0-7) and 24-31 (tile B)
# bottom half (64:128):       fields 8-15 (tile A) and 16-23 (tile B)


def _field_loc(i: int):
    if i < 8:
        return 0, i, 0        # tile A, slot i, top
    if i < 16:
        return 0, i - 8, 1    # tile A, bottom
    if i < 24:
        return 1, i - 16, 1   # tile B, bottom
    return 1, i - 24, 0       # tile B, top


_XTP_FIELD = np.zeros((2, 16), dtype=np.int64)   # [half, tile*8+slot] -> field
for _i in range(NF):
    _tl, _sl, _hf = _field_loc(_i)
    _XTP_FIELD[_hf, _tl * 8 + _sl] = _i

# ---- stage-1 units and W block packing -------------------------------------
# unit = (i, off, w): psum tile of w<=1024 T columns for first-field i
_UNITS_TOP = []
_UNITS_BOT = []
for _i in range(NF - 1):
    _ncols = (NF - 1 - _i) * K
    _lst = _UNITS_TOP if _field_loc(_i)[2] == 0 else _UNITS_BOT
    for _off in range(0, _ncols, UNIT):
        _lst.append((_i, _off, min(UNIT, _ncols - _off)))
_UNITS_TOP.sort(key=lambda c: (-c[2], c[0], c[1]))
_UNITS_BOT.sort(key=lambda c: (-c[2], c[0], c[1]))
assert len(_UNITS_TOP) == len(_UNITS_BOT)
_NBLK = len(_UNITS_TOP)

# W blocks: block j pairs _UNITS_TOP[j] (rows 0:64) with _UNITS_BOT[j]
# (rows 64:128). Width multisets match exactly, so no padding.
_WBLK = []        # (top_unit, bot_unit, width)
_WCOL = []        # start col of block j in packed W
_c = 0
for _j in range(_NBLK):
    _w = max(_UNITS_TOP[_j][2], _UNITS_BOT[_j][2])
    _WBLK.append((_UNITS_TOP[_j], _UNITS_BOT[_j], _w))
    _WCOL.append(_c)
    _c += _w
_WCOLS = _c

# execution order: alternate top/bottom so stage-1 row-packing overlaps
_ORDER = []       # (unit, blk_idx, half)
for _j in range(_NBLK):
    _ORDER.append((_UNITS_TOP[_j], _j, 0))
    _ORDER.append((_UNITS_BOT[_j], _j, 1))

# ---- routing and RED regions (tunable) -------------------------------------
# route per unit: 'p' (DVE from psum), 'ag' (ACT copy + GP mul),
#                 'a' (ACT copy + DVE mul)
# default: top units 'p', bottom units 'ag'
_ROUTE = {}
for _u in _UNITS_TOP:
    _ROUTE[_u] = "p"
for _u in _UNITS_BOT:
    _ROUTE[_u] = "a" if _NOGPMUL else "ag"
# offload a few top units to ACT-copy + DVE-bf16-mul (DVE is the
# saturated engine; the bf16 2x mode halves its per-col cost)
for _k in range(_A_UNITS):
    _ROUTE[_UNITS_TOP[8 + _k]] = "a"

# RED regions by field boundary. PE region must cover exactly the top pairs
# (their M columns are written by DVE 'p' MULs whose psum wait subsumes the
# cross-b-tile WAR on PE).
# top pairs: fields 0-7 -> pairs 0..219, fields 24-30 -> pairs 468..495.
_DV2GP = os.environ.get("V2_DV2GP", "1") == "1"
_DG = 123   # DVE-tree pairs total (fields 8-13)
_GG = 182 if _DV2GP else 125   # GP-tree pairs total

# emission indexes where each RED becomes ready: last ORDER index covering
# the region's fields
_last_cover = {}
for _idx, ((_i, _off, _w), _j, _hf) in enumerate(_ORDER):
    _last_cover[_i] = _idx


def _region_ready(fields):
    return max(_last_cover[f] for f in fields if f in _last_cover)


# RED plan: (ready_idx, kind, pair0, npairs, dstcol/scr_off)
#  kind 'pe': ident tile into acc_ps at col dstcol
#  kind 'dv'/'gp': tree over pairs [pair0, pair0+npairs) using the engine's
#  scratch rows [scr_off, scr_off+npairs)
if _SPLITRED:
    _RED_PLAN = [
        (_region_ready(range(0, 5)), "pe", 0, 124, 0),
        (_region_ready(range(5, 8)), "pe", 124, 96, 124),
        (_region_ready(range(24, 31)), "pe", 468, 28, 220),
        (_region_ready(range(8, 11)), "dv", 220, 66, 0),
        (_region_ready(range(11, 14)), "pe" if _TREE2PE else "dv", 286, 57,
         248 if _TREE2PE else 66),
        (_region_ready(range(14, 19)), "gp", 343, 75, 0),
        (_region_ready(range(19, 24)), "gp", 418, 50, 75),
    ]
else:
    _RED_PLAN = [
        (_region_ready([i for i in range(NF - 1)
                        if _field_loc(i)[2] == 0]), "pe", 0, 124, 0),
        (_region_ready([i for i in range(NF - 1)
                        if _field_loc(i)[2] == 0]), "pe", 124, 96, 124),
        (_region_ready([i for i in range(NF - 1)
                        if _field_loc(i)[2] == 0]), "pe", 468, 28, 220),
        (_region_ready(range(8, 14)),
         "gp" if _DV2GP else ("pe" if _TREE2PE else "dv"), 286, 57,
         125 if _DV2GP else (248 if _TREE2PE else 66)),
        (_region_ready(range(8, 14)), "dv", 220, 66, 0),
        (_region_ready(range(14, 24)), "dv" if _NOGPTREE else "gp", 343, 75, 0),
        (_region_ready(range(14, 24)), "dv" if _NOGPTREE else "gp", 418, 50, 75),
    ]
_RED_PLAN = sorted(_RED_PLAN, key=lambda r: r[0])


def _build_module() -> bass.Bass:
    nc = bass.Bass()
    xn = nc.declare_dram_parameter("xn", [BC, NF * K], BF16, isOutput=False)
    xtp = nc.declare_dram_parameter("xtp", [BT, 16, BC], BF16, isOutput=False)
    wt = nc.declare_dram_parameter("wt", [BT, _WCOLS], BF16, isOutput=False)
    ident = nc.declare_dram_parameter("ident", [BT, BT], BF16, isOutput=False)
    outs = [
        nc.declare_dram_parameter(f"out{t}", [BT, P], F32, isOutput=True)
        for t in range(NBT)
    ]

    with tile.TileContext(nc) as tc:
        with (
            tc.tile_pool(name="wpool", bufs=1) as wpool,
            tc.tile_pool(name="xnp", bufs=1) as xnp,
            tc.tile_pool(name="xtp_pool", bufs=1) as xtpool,
            tc.tile_pool(name="mpool", bufs=1) as mpool,
            tc.tile_pool(name="tbp", bufs=_TB_BUFS) as tbp,
            tc.tile_pool(name="scrp", bufs=1) as scrp,
            tc.tile_pool(name="accp", bufs=1) as accp,
            tc.tile_pool(name="ups", bufs=(3 if UNIT == 1024 else 6), space=bass.MemorySpace.PSUM) as ups,
            tc.tile_pool(name="accps", bufs=1, space=bass.MemorySpace.PSUM) as accpsp,
            tc.tile_pool(name="sinkp", bufs=1, space=bass.MemorySpace.PSUM) as sinkp,
        ):
            sink = sinkp.tile([BT, 8], F32)
            junk = wpool.tile([BT, 256], F32, tag="junk")
            junka = wpool.tile([BT, 256], F32, tag="junka")
            junkg = wpool.tile([BT, 256], F32, tag="junkg")
            _tcnt = {"d": 0, "a": 0, "g": 0}

            _NLT = os.environ.get("V2_NO_LOAD_TOUCH", "0") == "1"

            def pe_touch(ap2d):
                if _NLT:
                    return
                # 1-col bf16 matmuls crash the NEFF on real TRN2; touch via
                # an f32 view of the same bytes instead
                a = (ap2d[:, 0:2].bitcast(F32)
                     if ap2d.dtype == BF16 else ap2d[:, 0:1])
                nc.tensor.matmul(
                    sink[0:1, 0:1], a[:, 0:1], a[:, 0:1],
                    start=True, stop=True,
                )

            def dve_touch(ap2d):
                c = _tcnt["d"] % 256
                _tcnt["d"] += 1
                nc.vector.tensor_copy(junk[0:1, c:c + 1], ap2d[0:1, 0:1])

            def act_touch(ap2d):
                c = _tcnt["a"] % 256
                _tcnt["a"] += 1
                nc.scalar.copy(junka[0:1, c:c + 1], ap2d[0:1, 0:1])

            def gp_touch(ap2d):
                c = _tcnt["g"] % 256
                _tcnt["g"] += 1
                nc.gpsimd.tensor_copy(junkg[0:1, c:c + 1], ap2d[0:1, 0:1])

            # persistent M buffer [BT, P*K] bf16, reused across b-tiles
            M = mpool.tile([BT, P * K], BF16, tag="M")
            M3 = M[:].rearrange("p (a b) -> p a b", b=K)
            if _MAXU < 999 and os.environ.get("V2_MEMSET", "1") == "1":  # M bf16 memset
                nc.vector.memset(M[:], 0.0)

            # tree scratch, one set per engine, reused across b-tiles
            _dgr = 248 if _NOGPTREE else _DG
            scrD = [
                scrp.tile([BT, _dgr, w], BF16, tag=f"scrD{w}", name=f"scrD{w}")
                for w in (32, 16, 8, 4, 2)
            ]
            scrG = [
                scrp.tile([BT, _GG, w], BF16, tag=f"scrG{w}", name=f"scrG{w}")
                for w in (32, 16, 8, 4, 2)
            ]
            treedump = [scrp.tile([BT, 256], F32, tag="treedump",
                                  name="treedump")]

            # ---- loads -----------------------------------------------------
            xn_tiles = [None] * NBT
            xt_tiles = [None] * NBT

            def load_xn(t, q=nc.sync):
                xn_sb = xnp.tile([BT, NF * K], BF16, tag=f"xn{t}", name="xn_sb")
                q.dma_start(xn_sb[:], xn[t * BT:(t + 1) * BT, :])
                dve_touch(xn_sb)
                gp_touch(xn_sb)
                xn_tiles[t] = xn_sb

            def load_xt(t, tl, q=nc.sync):
                xg = xtpool.tile(
                    [BT, 8, BT], BF16, tag=f"xt{t}_{tl}", name="xg"
                )
                q.dma_start(
                    xg[:], xtp[:, 8 * tl: 8 * (tl + 1), t * BT:(t + 1) * BT]
                )
                pe_touch(xg[0:64, 0, :])
                pe_touch(xg[64:128, 0, :])
                if xt_tiles[t] is None:
                    xt_tiles[t] = [None, None]
                xt_tiles[t][tl] = xg

            w_tiles = [None] * len(_WBLK)

            def load_w(j):
                ct, cb, w = _WBLK[j]
                wtile = wpool.tile([BT, w], BF16, tag=f"w{j}", name="wtile")
                nc.sync.dma_start(wtile[:], wt[:, _WCOL[j]: _WCOL[j] + w])
                pe_touch(wtile[0:64, :])
                pe_touch(wtile[64:128, :])
                w_tiles[j] = wtile

            ident_sb = wpool.tile([BT, BT], BF16, tag="ident")

            # b-tile 0's working set races the first compute on the sync
            # queue; everything else streams on the scalar-engine queue.
            load_xt(0, 0)
            load_w(0)
            load_xn(0)
            load_xt(0, 1)
            nc.sync.dma_start(ident_sb[:], ident[:])
            pe_touch(ident_sb)
            if not _INLOOP_LD:
                for j in range(1, 6):
                    load_w(j)
                load_xn(1)
                load_xt(1, 0)
                load_xt(1, 1)
                for j in range(6, 13):
                    load_w(j)
                load_xn(2)
                load_xt(2, 0)
                load_xt(2, 1)
                for j in range(13, 19):
                    load_w(j)
                load_xn(3)
                load_xt(3, 0)
                load_xt(3, 1)
                for j in range(19, len(_WBLK)):
                    load_w(j)
            else:
                for j in range(1, min(_MAXW, len(_WBLK))):
                    load_w(j)

            # ---- compute ---------------------------------------------------
            def emit_tree(add_fn, scr, so, p0, G, dst):
                src = M3[:, p0:p0 + G, :]
                s = [sc[:, so:so + G, :] for sc in scr]
                steps = [
                    lambda: add_fn(s[0], src[:, :, 0:32], src[:, :, 32:64]),
                    lambda: add_fn(s[1], s[0][:, :, 0:16], s[0][:, :, 16:32]),
                    lambda: add_fn(s[2], s[1][:, :, 0:8], s[1][:, :, 8:16]),
                    lambda: add_fn(s[3], s[2][:, :, 0:4], s[2][:, :, 4:8]),
                    lambda: add_fn(s[4], s[3][:, :, 0:2], s[3][:, :, 2:4]),
                ]
                with nc.allow_low_precision(reason="bf16 tree; 2e-2 gate"):
                    for _k in range(min(_TDEPTH, 5)):
                        steps[_k]()
                if _TDEPTH >= 6:
                    if os.environ.get("V2_TREE_DUMMY", "0") == "1":
                        dst = treedump[0][:, :dst.shape[1]]
                    out3 = dst.rearrange("p (a o) -> p a o", o=1)
                    add_fn(out3, s[4][:, :, 0:1], s[4][:, :, 1:2])

            acc_sb0 = [None]
            for t in range(NBT):
                if _ONLY_T0 and t > 0:
                    if os.environ.get("V2_SKIP_OUT123", "0") != "1":
                        if _SYNC_OUT:
                            nc.sync.dma_start(outs[t][:], acc_sb0[0][:])
                        else:
                            nc.gpsimd.dma_start(outs[t][:], acc_sb0[0][:])
                    continue
                xn_sb = xn_tiles[t]
                xt_sb = xt_tiles[t]

                acc_ps = accpsp.tile([BT, 512], F32, tag="acc", name="acc_ps")
                acc_sb = scrp.tile([BT, P], F32, tag=f"accsb{t}", name="acc_sb")
                acc_sb0[0] = acc_sb

                copies = 0
                last_gp_mcol = [None]      # recent GP-written M elem
                cons_hist = []             # per-unit consumer loc (psum WAR)

                for idx, (u, j, half) in enumerate(_ORDER):
                    # stream the next b-tile's inputs on the sync queue
                    # (empty once the W blocks are in)
                    if _INLOOP_LD and t + 1 < NBT and idx in (12, 22, 32):
                        if idx == 12:
                            load_xn(t + 1)
                        elif idx == 22:
                            load_xt(t + 1, 0)
                        else:
                            load_xt(t + 1, 1)
                    if idx >= _MAXU:
                        continue
                    i, off, w = u
                    tl, slot, hf = _field_loc(i)
                    assert hf == half
                    pb = 64 * half
                    # absorb the psum WAR tick (consumer of unit idx-3) into
                    # PE's clock so the S1 matmuls keep only their self-wait
                    if _PETOUCH and len(cons_hist) >= 3:
                        pe_touch(cons_hist[-3])
                    ps = ups.tile([BT, UNIT], F32, name="ps")
                    for s in range(0, w, 512):
                        sw = min(512, w - s)
                        nc.tensor.matmul(
                            ps[:, s:s + sw],
                            xt_sb[tl][pb:pb + 64, slot, :],
                            w_tiles[j][pb:pb + 64, s:s + sw],
                            start=True,
                            stop=True,
                        )
                    mcol = _GRP_OFF[i] * K + off
                    c1 = (i + 1) * K + off
                    route = _ROUTE[u]
                    if route == "p":
                        nc.vector.tensor_mul(
                            M[:, mcol:mcol + w], ps[:, :w], xn_sb[:, c1:c1 + w]
                        )
                        cons_hist.append(M[:, mcol:mcol + 1])
                    else:
                        tb = tbp.tile([BT, UNIT], BF16, name="tb")
                        if (_ACTMT and copies % 3 == 2
                                and last_gp_mcol[0] is not None):
                            act_touch(M[:, last_gp_mcol[0]:last_gp_mcol[0] + 1])
                        copies += 1
                        nc.scalar.copy(tb[:, :w], ps[:, :w])
                        cons_hist.append(tb[:, 0:1])
                        if route == "ag":
                            if _AGTOUCH:
                                gp_touch(tb)
                            nc.gpsimd.tensor_mul(
                                M[:, mcol:mcol + w], tb[:, :w],
                                xn_sb[:, c1:c1 + w],
                            )
                            last_gp_mcol[0] = mcol
                        else:
                            if _AGTOUCH:
                                dve_touch(tb)
                            nc.vector.tensor_mul(
                                M[:, mcol:mcol + w], tb[:, :w],
                                xn_sb[:, c1:c1 + w],
                            )

                    for (ridx, kind, p0, npair, aux) in _RED_PLAN:
                        if ridx != idx or _NORED or kind not in _RED_KINDS:
                            continue
                        if kind == "pe":
                            # absorb the M RAW tick (MUL engine) so the
                            # ident matmuls keep only their psum self-wait
                            pe_touch(M[:, p0 * K:p0 * K + 2])
                            for l in range(_NIDENT):
                                nc.tensor.matmul(
                                    acc_ps[:, aux:aux + npair],
                                    ident_sb[:],
                                    M3[:, p0:p0 + npair, l],
                                    start=(l == 0),
                                    stop=(l == _NIDENT - 1),
                                )
                        elif kind == "dv":
                            emit_tree(
                                nc.vector.tensor_add, scrD, aux,
                                p0, npair, acc_sb[:, p0:p0 + npair],
                            )
                        else:
                            emit_tree(
                                nc.gpsimd.tensor_add, scrG, aux,
                                p0, npair, acc_sb[:, p0:p0 + npair],
                            )

                # drain PE-region psum accumulators -> acc_sb
                if _NORED or _NODRAIN:
                    if os.environ.get("V2_MEMSET", "1") == "1" and _NORED:
                        nc.vector.memset(acc_sb[:], 0.0)
                else:
                    for (_r, _kind, _p0, _np, _aux) in _RED_PLAN:
                        if _kind == "pe" and "pe" in _RED_KINDS:
                            nc.vector.tensor_copy(
                                acc_sb[:, _p0:_p0 + _np],
                                acc_ps[:, _aux:_aux + _np],
                            )

                pk = os.environ.get("V2_POKE_ACCSB", "0")
                if pk == "1":
                    nc.vector.tensor_copy(acc_sb[0:1, 0:1], junk[0:1, 0:1])
                elif pk == "act":
                    nc.scalar.copy(acc_sb[0:1, 0:1], junk[0:1, 0:1])
                elif pk == "gp":
                    nc.gpsimd.tensor_copy(acc_sb[0:1, 0:1], junk[0:1, 0:1])
                elif pk == "full":
                    nc.vector.tensor_copy(acc_sb[:], junk[:, 0:1].broadcast(1, P))
                # absorb DVE's acc_sb writes so the DMA keeps <=1 wait
                if _SYNC_OUT:
                    nc.sync.dma_start(outs[t][:], acc_sb[:])
                else:
                    if os.environ.get("V2_NO_ACCSB_TOUCH", "0") != "1":
                        gp_touch(acc_sb[:, 0:1])
                    nc.gpsimd.dma_start(outs[t][:], acc_sb[:])

    _split_multi_waits(nc)
    _fix_dma_waits(nc)
    return nc


_NC_CACHE: dict[str, bass.Bass] = {}


def _get_module() -> bass.Bass:
    if "nc" not in _NC_CACHE:
        _NC_CACHE["nc"] = _build_module()
    return _NC_CACHE["nc"]


def _make_in_maps(inputs: np.ndarray, W: np.ndarray):
    import ml_dtypes

    x = np.ascontiguousarray(np.asarray(inputs, dtype=np.float32)[:, :, 0, :])
    W = np.asarray(W, dtype=np.float32)

    # packed W: block j = [top unit | bottom unit] on partition halves
    wt_host = np.zeros((BT, _WCOLS), dtype=ml_dtypes.bfloat16)
    wt_flat = np.ascontiguousarray(W.transpose(1, 0, 2)).reshape(K, P * K)
    wt_flat = wt_flat.astype(ml_dtypes.bfloat16)
    for j, (ct, cb, w) in enumerate(_WBLK):
        for half, (i, off, cw) in ((0, ct), (1, cb)):
            base = _GRP_OFF[i] * K + off
            wt_host[64 * half: 64 * half + 64, _WCOL[j]: _WCOL[j] + cw] = \
                wt_flat[:, base: base + cw]

    ident = np.eye(BT, dtype=ml_dtypes.bfloat16)

    in_maps = []
    for c in range(NCORES):
        xs = x[:, c * BC:(c + 1) * BC, :]                      # (32, 512, 64)
        xn_host = np.ascontiguousarray(
            xs.transpose(1, 0, 2)
        ).reshape(BC, NF * K).astype(ml_dtypes.bfloat16)
        # xtp[p, slot16, b]: p<64 top fields, p>=64 bottom fields, k = p % 64
        xtp_host = np.empty((BT, 16, BC), dtype=ml_dtypes.bfloat16)
        xt_all = xs.transpose(2, 0, 1).astype(ml_dtypes.bfloat16)  # (64,32,512)
        xtp_host[0:64] = xt_all[:, _XTP_FIELD[0], :]
        xtp_host[64:128] = xt_all[:, _XTP_FIELD[1], :]
        in_maps.append(
            {"xn": xn_host, "xtp": xtp_host, "wt": wt_host, "ident": ident}
        )
    return in_maps


def kernel(inputs: np.ndarray, W: np.ndarray) -> np.ndarray:
    in_maps = _make_in_maps(inputs, W)
    nc = _get_module()
    res = run_bass_kernel_spmd(nc, in_maps, list(range(NCORES))).results
    return np.concatenate(
        [r[f"out{t}"] for r in res for t in range(NBT)], axis=0
    )


def kernel_profiled(inputs: np.ndarray, W: np.ndarray, tmpdir: str | None = None):
    """Run with NTFF tracing; returns (output, BassKernelResults)."""
    in_maps = _make_in_maps(inputs, W)
    nc = _get_module()
    br = run_bass_kernel_spmd(
        nc, in_maps, list(range(NCORES)), trace=True, tmpdir=tmpdir
    )
    out = np.concatenate(
        [r[f"out{t}"] for r in br.results for t in range(NBT)], axis=0
    )
    return out, br
